# revision 2
# baseline (speedup 1.0000x reference)
"""EdgeEmbedding kernel for 8 Trainium2 NeuronCores, v3.

y[e] = silu(concat(h[src[e]], h[tgt[e]], m[e]) @ W) / 0.6
     = silu(T1[src] + T2[tgt] + m @ W3) / 0.6,  T1|T2 = h @ [W1|W2]

v3: replaces the per-128-descriptor indirect_dma_start gathers (Pool
engine 65% busy on ~1us/call SWDGE fixed cost in the baseline) with
dma_gather (InstDMAGatherAnt): 1024 indices per call, 4 calls per
2048-edge group. int16 indices require <=32768-row table windows, so
edges are bucketed host-side by (src_chunk, tgt_chunk) over 4 chunks of
25088 atoms; each bucket has a static quota of 8 groups (16384 edges),
padded with index-0 edges. Within a bucket edges are Morton-sorted for
DRAM row locality. The host unpermutes the output rows.

Pair table Tp[a] = [T1[a] | T2[a]] (f32, [100352, 128]) is built on
device (phase 1) and gathered with elem_step=128: src reads cols 0:64
of row src_local, tgt reads cols 64:128. h/W/m are fed bf16 (f32 psum).
"""

import numpy as np

import concourse.bass as bass
import concourse.mybir as mybir
from concourse import bacc
from concourse.tile import TileContext
from concourse.bass_utils import run_bass_kernel_spmd

N_CORES = 8
NUM_ATOMS = 100000
CH_A = 25088              # atoms per chunk
NCH = 4                   # chunks
A2 = CH_A * NCH           # 100352 padded atoms
E_CORE = 250000
NB = 16                   # buckets = src_chunk * 4 + tgt_chunk
BG = 8                    # groups per bucket
QUOTA = BG * 2048         # 16384 edges per bucket
NG2 = NB * BG             # 128 groups per core
E_DEV = NG2 * 2048        # 262144 slots
SCALE = 1.0 / 0.6
F32 = mybir.dt.float32
BF16 = mybir.dt.bfloat16
I16 = mybir.dt.int16
NP_BF16 = mybir.dt.np(BF16)

_PROG = None


def _build_program():
    nc = bacc.Bacc("TRN2", target_bir_lowering=False, debug=False)
    hT = nc.dram_tensor("hT", [64, A2], BF16, kind="ExternalInput")
    Wcat = nc.dram_tensor("Wcat", [64, 128], BF16, kind="ExternalInput")
    W3 = nc.dram_tensor("W3", [16, 64], BF16, kind="ExternalInput")
    idxw = nc.dram_tensor("idxw", [NG2, 128, 4, 64], I16, kind="ExternalInput")
    m_st = nc.dram_tensor("m_st", [NG2, 16, 2048], BF16, kind="ExternalInput")
    out = nc.dram_tensor("out", [NG2, 128, 16, 64], F32, kind="ExternalOutput")

    with TileContext(nc) as tc:
        with tc.tile_pool(name="dram", bufs=1, space="DRAM") as dpool:
            Tp = dpool.tile([A2, 128], F32)

            # ---- phase 1: pair table, 784 blocks of 128 atoms ----
            with tc.tile_pool(name="ph1", bufs=2) as p1, \
                 tc.tile_pool(name="cp", bufs=3) as cp, \
                 tc.tile_pool(name="ps1", bufs=4, space="PSUM") as ps1, \
                 tc.tile_pool(name="wp", bufs=1) as wp:
                wcat_sb = wp.tile([64, 128], BF16)
                nc.sync.dma_start(wcat_sb[:, :], Wcat[:, :])
                for c in range(A2 // 2048):
                    hTc = p1.tile([64, 2048], BF16, tag="hTc")
                    nc.sync.dma_start(hTc[:, :], hT[:, 2048 * c:2048 * (c + 1)])
                    for b in range(16):
                        ps = ps1.tile([128, 128], F32)
                        nc.tensor.matmul(
                            out=ps[:, :],
                            lhsT=hTc[:, 128 * b:128 * b + 128],
                            rhs=wcat_sb[:, :],
                            start=True, stop=True)
                        t12 = cp.tile([128, 128], F32, tag="t12")
                        nc.scalar.copy(t12[:, :], ps[:, :])
                        a0 = 2048 * c + 128 * b
                        nc.sync.dma_start(Tp[a0:a0 + 128, :], t12[:, :])

            tc.strict_bb_all_engine_barrier()

            # ---- phase 2: 128 groups of 2048 bucket-sorted edges ----
            with tc.tile_pool(name="ip", bufs=3) as ip, \
                 tc.tile_pool(name="mp", bufs=3) as mp, \
                 tc.tile_pool(name="gp", bufs=2) as gp, \
                 tc.tile_pool(name="vp", bufs=4) as vp, \
                 tc.tile_pool(name="op", bufs=2) as op, \
                 tc.tile_pool(name="ps2", bufs=4, space="PSUM") as ps2, \
                 tc.tile_pool(name="wp2", bufs=1) as wp2:
                w3_sb = wp2.tile([16, 64], BF16)
                nc.sync.dma_start(w3_sb[:, :], W3[:, :])
                for t in range(NG2):
                    bs, bt = (t // BG) >> 2, (t // BG) & 3
                    src_tab = Tp[CH_A * bs:CH_A * (bs + 1), 0:64]
                    tgt_tab = Tp[CH_A * bt:CH_A * (bt + 1), 64:128]
                    it = ip.tile([128, 4, 64], I16, tag="it")
                    nc.sync.dma_start(it[:, :, :], idxw[t])
                    mst = mp.tile([16, 2048], BF16, tag="mst")
                    nc.sync.dma_start(mst[:, :], m_st[t])
                    gs = gp.tile([128, 16, 64], F32, tag="gs")
                    gt2 = gp.tile([128, 16, 64], F32, tag="gt")
                    for half in range(2):
                        nc.gpsimd.dma_gather(
                            out_ap=gs[:, 8 * half:8 * half + 8, :],
                            in_ap=src_tab,
                            idxs_ap=it[:, half, :],
                            num_idxs=1024, num_idxs_reg=1024,
                            elem_size=64, elem_step=128,
                            transpose=False)
                        nc.gpsimd.dma_gather(
                            out_ap=gt2[:, 8 * half:8 * half + 8, :],
                            in_ap=tgt_tab,
                            idxs_ap=it[:, 2 + half, :],
                            num_idxs=1024, num_idxs_reg=1024,
                            elem_size=64, elem_step=128,
                            transpose=False)
                    ot = op.tile([128, 16, 64], F32, tag="ot")
                    for b in range(8):
                        ps = ps2.tile([128, 2, 64], F32)
                        for hh in range(2):
                            j = 2 * b + hh
                            nc.tensor.matmul(
                                out=ps[:, hh, :],
                                lhsT=mst[:, 128 * j:128 * j + 128],
                                rhs=w3_sb[:, :],
                                start=True, stop=True)
                        y = vp.tile([128, 2, 64], F32, tag="y")
                        nc.vector.tensor_tensor(
                            out=y[:, :, :], in0=gs[:, 2 * b:2 * b + 2, :],
                            in1=gt2[:, 2 * b:2 * b + 2, :],
                            op=mybir.AluOpType.add)
                        nc.vector.tensor_tensor(
                            out=y[:, :, :], in0=y[:, :, :], in1=ps[:, :, :],
                            op=mybir.AluOpType.add)
                        z = vp.tile([128, 2, 64], F32, tag="z")
                        nc.scalar.activation(
                            out=z[:, :, :], in_=y[:, :, :],
                            func=mybir.ActivationFunctionType.Silu)
                        nc.vector.tensor_scalar_mul(
                            ot[:, 2 * b:2 * b + 2, :], z[:, :, :], SCALE)
                    nc.scalar.dma_start(out[t], ot[:, :, :])
    nc.finalize()
    return nc


def _part1by1(x):
    x = x.astype(np.uint32)
    x &= 0x0000ffff
    x = (x | (x << 8)) & 0x00ff00ff
    x = (x | (x << 4)) & 0x0f0f0f0f
    x = (x | (x << 2)) & 0x33333333
    x = (x | (x << 1)) & 0x55555555
    return x


def _morton(x, y):
    return _part1by1(x) | (_part1by1(y) << 1)


def _wrap_idx(vals):
    """[NG2*2048] int16 -> [NG2, 128, 2, 64]: per 1024-idx call, index k
    lives at partition k%16 (replicated x8), free position k//16."""
    v = vals.reshape(NG2, 2, 64, 16)           # [t, call, fpos, pmod]
    v = v.transpose(0, 3, 1, 2)                # [t, pmod, call, fpos]
    return np.tile(v, (1, 8, 1, 1))            # [t, 128, 2, 64]


def _prepare_inputs(h, m, edge_index, W):
    h = np.asarray(h, dtype=np.float32)
    m = np.asarray(m, dtype=np.float32)
    W = np.asarray(W, dtype=np.float32)
    ei = np.asarray(edge_index).astype(np.int64)

    hT = np.zeros((64, A2), dtype=NP_BF16)
    hT[:, :NUM_ATOMS] = h.T.astype(NP_BF16)
    Wcat = np.concatenate([W[0:64, :], W[64:128, :]], axis=1).astype(NP_BF16)
    W3 = W[128:144, :].astype(NP_BF16)

    in_maps = []
    perms = []
    for c in range(N_CORES):
        lo = c * E_CORE
        src = ei[0, lo:lo + E_CORE]
        tgt = ei[1, lo:lo + E_CORE]
        bs = src // CH_A
        bt = tgt // CH_A
        bucket = bs * 4 + bt
        ls = (src - bs * CH_A).astype(np.int16)
        lt = (tgt - bt * CH_A).astype(np.int16)
        zkey = _morton(ls.astype(np.uint32), lt.astype(np.uint32))
        order = np.lexsort((zkey, bucket))
        counts = np.bincount(bucket, minlength=NB)
        if counts.max() > QUOTA:
            raise RuntimeError(f"bucket overflow: {counts.max()} > {QUOTA}")
        starts = np.concatenate([[0], np.cumsum(counts)])

        perm = np.full(E_DEV, -1, dtype=np.int64)   # slot -> edge id (core-local)
        ls_s = np.zeros(E_DEV, dtype=np.int16)
        lt_s = np.zeros(E_DEV, dtype=np.int16)
        m_s = np.zeros((E_DEV, 16), dtype=np.float32)
        for b in range(NB):
            ids = order[starts[b]:starts[b + 1]]
            base = b * QUOTA
            perm[base:base + len(ids)] = ids
            ls_s[base:base + len(ids)] = ls[ids]
            lt_s[base:base + len(ids)] = lt[ids]
            m_s[base:base + len(ids)] = m[lo + ids]

        idxw = np.concatenate([_wrap_idx(ls_s), _wrap_idx(lt_s)], axis=2)
        mst = np.ascontiguousarray(
            m_s.reshape(NG2, 16, 128, 16).transpose(0, 3, 1, 2)
               .reshape(NG2, 16, 2048)).astype(NP_BF16)
        in_maps.append({"hT": hT, "Wcat": Wcat, "W3": W3,
                        "idxw": np.ascontiguousarray(idxw), "m_st": mst})
        perms.append(perm)
    return in_maps, perms


def _run(inputs, trace=False):
    global _PROG
    if _PROG is None:
        _PROG = _build_program()
    in_maps, perms = _prepare_inputs(**inputs)
    res = run_bass_kernel_spmd(
        _PROG, in_maps, core_ids=list(range(N_CORES)), trace=trace)
    full = np.empty((N_CORES * E_CORE, 64), dtype=np.float32)
    for c in range(N_CORES):
        o = res.results[c]["out"]  # [NG2, 128, 16, 64]
        o = o.transpose(0, 2, 1, 3).reshape(E_DEV, 64)
        perm = perms[c]
        valid = perm >= 0
        full[c * E_CORE + perm[valid]] = o[valid]
    return full, res


def kernel(h, m, edge_index, W):
    full, _ = _run(dict(h=h, m=m, edge_index=edge_index, W=W), trace=False)
    return full


# revision 3
# speedup vs baseline: 1.2024x; 1.2024x over previous
"""EdgeEmbedding kernel for 8 Trainium2 NeuronCores, v3.

y[e] = silu(concat(h[src[e]], h[tgt[e]], m[e]) @ W) / 0.6
     = silu(T1[src] + T2[tgt] + m @ W3) / 0.6,  T1|T2 = h @ [W1|W2]

v3: replaces the per-128-descriptor indirect_dma_start gathers (Pool
engine 65% busy on ~1us/call SWDGE fixed cost in the baseline) with
dma_gather (InstDMAGatherAnt): 1024 indices per call, 4 calls per
2048-edge group. int16 indices require <=32768-row table windows, so
edges are bucketed host-side by (src_chunk, tgt_chunk) over 4 chunks of
25088 atoms; each bucket has a static quota of 8 groups (16384 edges),
padded with index-0 edges. Within a bucket edges are Morton-sorted for
DRAM row locality. The host unpermutes the output rows.

Pair table Tp[a] = [T1[a] | T2[a]] (f32, [100352, 128]) is built on
device (phase 1) and gathered with elem_step=128: src reads cols 0:64
of row src_local, tgt reads cols 64:128. h/W/m are fed bf16 (f32 psum).
"""

import numpy as np

import concourse.bass as bass
import concourse.mybir as mybir
from concourse import bacc
from concourse.tile import TileContext
from concourse.bass_utils import run_bass_kernel_spmd

N_CORES = 8
NUM_ATOMS = 100000
CH_A = 25088              # atoms per chunk
NCH = 4                   # chunks
A2 = CH_A * NCH           # 100352 padded atoms
E_CORE = 250000
NB = 16                   # buckets = src_chunk * 4 + tgt_chunk
BG = 8                    # groups per bucket
QUOTA = BG * 2048         # 16384 edges per bucket
NG2 = NB * BG             # 128 groups per core
E_DEV = NG2 * 2048        # 262144 slots
SCALE = 1.0 / 0.6
F32 = mybir.dt.float32
BF16 = mybir.dt.bfloat16
I16 = mybir.dt.int16
NP_BF16 = mybir.dt.np(BF16)

_PROG = None


def _build_program():
    nc = bacc.Bacc("TRN2", target_bir_lowering=False, debug=False,
                   num_swdge_queues=4)
    hT = nc.dram_tensor("hT", [64, A2], BF16, kind="ExternalInput")
    Wcat = nc.dram_tensor("Wcat", [64, 128], BF16, kind="ExternalInput")
    W3 = nc.dram_tensor("W3", [16, 64], BF16, kind="ExternalInput")
    idxw = nc.dram_tensor("idxw", [NG2, 128, 4, 64], I16, kind="ExternalInput")
    m_st = nc.dram_tensor("m_st", [NG2, 16, 2048], BF16, kind="ExternalInput")
    out = nc.dram_tensor("out", [NG2, 128, 16, 64], F32, kind="ExternalOutput")

    with TileContext(nc) as tc:
        with tc.tile_pool(name="dram", bufs=1, space="DRAM") as dpool:
            Tp = dpool.tile([A2, 128], F32)

            # ---- phase 1: pair table, 784 blocks of 128 atoms ----
            with tc.tile_pool(name="ph1", bufs=2) as p1, \
                 tc.tile_pool(name="cp", bufs=3) as cp, \
                 tc.tile_pool(name="ps1", bufs=4, space="PSUM") as ps1, \
                 tc.tile_pool(name="wp", bufs=1) as wp:
                wcat_sb = wp.tile([64, 128], BF16)
                nc.sync.dma_start(wcat_sb[:, :], Wcat[:, :])
                for c in range(A2 // 2048):
                    hTc = p1.tile([64, 2048], BF16, tag="hTc")
                    nc.sync.dma_start(hTc[:, :], hT[:, 2048 * c:2048 * (c + 1)])
                    for b in range(16):
                        ps = ps1.tile([128, 128], F32)
                        nc.tensor.matmul(
                            out=ps[:, :],
                            lhsT=hTc[:, 128 * b:128 * b + 128],
                            rhs=wcat_sb[:, :],
                            start=True, stop=True)
                        t12 = cp.tile([128, 128], F32, tag="t12")
                        nc.scalar.copy(t12[:, :], ps[:, :])
                        a0 = 2048 * c + 128 * b
                        nc.sync.dma_start(Tp[a0:a0 + 128, :], t12[:, :])

            tc.strict_bb_all_engine_barrier()

            # ---- phase 2: 128 groups of 2048 bucket-sorted edges ----
            with tc.tile_pool(name="ip", bufs=3) as ip, \
                 tc.tile_pool(name="mp", bufs=3) as mp, \
                 tc.tile_pool(name="gp", bufs=2) as gp, \
                 tc.tile_pool(name="vp", bufs=4) as vp, \
                 tc.tile_pool(name="op", bufs=2) as op, \
                 tc.tile_pool(name="ps2", bufs=4, space="PSUM") as ps2, \
                 tc.tile_pool(name="wp2", bufs=1) as wp2:
                w3_sb = wp2.tile([16, 64], BF16)
                nc.sync.dma_start(w3_sb[:, :], W3[:, :])
                for t in range(NG2):
                    bs, bt = (t // BG) >> 2, (t // BG) & 3
                    src_tab = Tp[CH_A * bs:CH_A * (bs + 1), 0:64]
                    tgt_tab = Tp[CH_A * bt:CH_A * (bt + 1), 64:128]
                    it = ip.tile([128, 4, 64], I16, tag="it")
                    nc.sync.dma_start(it[:, :, :], idxw[t])
                    mst = mp.tile([16, 2048], BF16, tag="mst")
                    nc.sync.dma_start(mst[:, :], m_st[t])
                    gs = gp.tile([128, 16, 64], F32, tag="gs")
                    gt2 = gp.tile([128, 16, 64], F32, tag="gt")
                    for half in range(2):
                        nc.gpsimd.dma_gather(
                            out_ap=gs[:, 8 * half:8 * half + 8, :],
                            in_ap=src_tab,
                            idxs_ap=it[:, half, :],
                            num_idxs=1024, num_idxs_reg=1024,
                            elem_size=64, elem_step=128,
                            transpose=False, single_packet=False,
                            queue_num=(4 * t + 2 * half) % 4)
                        nc.gpsimd.dma_gather(
                            out_ap=gt2[:, 8 * half:8 * half + 8, :],
                            in_ap=tgt_tab,
                            idxs_ap=it[:, 2 + half, :],
                            num_idxs=1024, num_idxs_reg=1024,
                            elem_size=64, elem_step=128,
                            transpose=False, single_packet=False,
                            queue_num=(4 * t + 2 * half + 1) % 4)
                    ot = op.tile([128, 16, 64], F32, tag="ot")
                    for b in range(8):
                        ps = ps2.tile([128, 2, 64], F32)
                        for hh in range(2):
                            j = 2 * b + hh
                            nc.tensor.matmul(
                                out=ps[:, hh, :],
                                lhsT=mst[:, 128 * j:128 * j + 128],
                                rhs=w3_sb[:, :],
                                start=True, stop=True)
                        y = vp.tile([128, 2, 64], F32, tag="y")
                        nc.vector.tensor_tensor(
                            out=y[:, :, :], in0=gs[:, 2 * b:2 * b + 2, :],
                            in1=gt2[:, 2 * b:2 * b + 2, :],
                            op=mybir.AluOpType.add)
                        nc.vector.tensor_tensor(
                            out=y[:, :, :], in0=y[:, :, :], in1=ps[:, :, :],
                            op=mybir.AluOpType.add)
                        z = vp.tile([128, 2, 64], F32, tag="z")
                        nc.scalar.activation(
                            out=z[:, :, :], in_=y[:, :, :],
                            func=mybir.ActivationFunctionType.Silu)
                        nc.vector.tensor_scalar_mul(
                            ot[:, 2 * b:2 * b + 2, :], z[:, :, :], SCALE)
                    nc.scalar.dma_start(out[t], ot[:, :, :])
    nc.finalize()
    return nc


def _part1by1(x):
    x = x.astype(np.uint32)
    x &= 0x0000ffff
    x = (x | (x << 8)) & 0x00ff00ff
    x = (x | (x << 4)) & 0x0f0f0f0f
    x = (x | (x << 2)) & 0x33333333
    x = (x | (x << 1)) & 0x55555555
    return x


def _morton(x, y):
    return _part1by1(x) | (_part1by1(y) << 1)


def _wrap_idx(vals):
    """[NG2*2048] int16 -> [NG2, 128, 2, 64]: per 1024-idx call, index k
    lives at partition k%16 (replicated x8), free position k//16."""
    v = vals.reshape(NG2, 2, 64, 16)           # [t, call, fpos, pmod]
    v = v.transpose(0, 3, 1, 2)                # [t, pmod, call, fpos]
    return np.tile(v, (1, 8, 1, 1))            # [t, 128, 2, 64]


def _prepare_inputs(h, m, edge_index, W):
    h = np.asarray(h, dtype=np.float32)
    m = np.asarray(m, dtype=np.float32)
    W = np.asarray(W, dtype=np.float32)
    ei = np.asarray(edge_index).astype(np.int64)

    hT = np.zeros((64, A2), dtype=NP_BF16)
    hT[:, :NUM_ATOMS] = h.T.astype(NP_BF16)
    Wcat = np.concatenate([W[0:64, :], W[64:128, :]], axis=1).astype(NP_BF16)
    W3 = W[128:144, :].astype(NP_BF16)

    in_maps = []
    perms = []
    for c in range(N_CORES):
        lo = c * E_CORE
        src = ei[0, lo:lo + E_CORE]
        tgt = ei[1, lo:lo + E_CORE]
        bs = src // CH_A
        bt = tgt // CH_A
        bucket = bs * 4 + bt
        ls = (src - bs * CH_A).astype(np.int16)
        lt = (tgt - bt * CH_A).astype(np.int16)
        zkey = _morton(ls.astype(np.uint32), lt.astype(np.uint32))
        order = np.lexsort((zkey, bucket))
        counts = np.bincount(bucket, minlength=NB)
        if counts.max() > QUOTA:
            raise RuntimeError(f"bucket overflow: {counts.max()} > {QUOTA}")
        starts = np.concatenate([[0], np.cumsum(counts)])

        perm = np.full(E_DEV, -1, dtype=np.int64)   # slot -> edge id (core-local)
        ls_s = np.zeros(E_DEV, dtype=np.int16)
        lt_s = np.zeros(E_DEV, dtype=np.int16)
        m_s = np.zeros((E_DEV, 16), dtype=np.float32)
        for b in range(NB):
            ids = order[starts[b]:starts[b + 1]]
            base = b * QUOTA
            perm[base:base + len(ids)] = ids
            ls_s[base:base + len(ids)] = ls[ids]
            lt_s[base:base + len(ids)] = lt[ids]
            m_s[base:base + len(ids)] = m[lo + ids]

        idxw = np.concatenate([_wrap_idx(ls_s), _wrap_idx(lt_s)], axis=2)
        mst = np.ascontiguousarray(
            m_s.reshape(NG2, 16, 128, 16).transpose(0, 3, 1, 2)
               .reshape(NG2, 16, 2048)).astype(NP_BF16)
        in_maps.append({"hT": hT, "Wcat": Wcat, "W3": W3,
                        "idxw": np.ascontiguousarray(idxw), "m_st": mst})
        perms.append(perm)
    return in_maps, perms


def _run(inputs, trace=False):
    global _PROG
    if _PROG is None:
        _PROG = _build_program()
    in_maps, perms = _prepare_inputs(**inputs)
    res = run_bass_kernel_spmd(
        _PROG, in_maps, core_ids=list(range(N_CORES)), trace=trace)
    full = np.empty((N_CORES * E_CORE, 64), dtype=np.float32)
    for c in range(N_CORES):
        o = res.results[c]["out"]  # [NG2, 128, 16, 64]
        o = o.transpose(0, 2, 1, 3).reshape(E_DEV, 64)
        perm = perms[c]
        valid = perm >= 0
        full[c * E_CORE + perm[valid]] = o[valid]
    return full, res


def kernel(h, m, edge_index, W):
    full, _ = _run(dict(h=h, m=m, edge_index=edge_index, W=W), trace=False)
    return full


# revision 4
# speedup vs baseline: 1.3800x; 1.1477x over previous
"""EdgeEmbedding kernel for 8 Trainium2 NeuronCores, v3.

y[e] = silu(concat(h[src[e]], h[tgt[e]], m[e]) @ W) / 0.6
     = silu(T1[src] + T2[tgt] + m @ W3) / 0.6,  T1|T2 = h @ [W1|W2]

v3: replaces the per-128-descriptor indirect_dma_start gathers (Pool
engine 65% busy on ~1us/call SWDGE fixed cost in the baseline) with
dma_gather (InstDMAGatherAnt): 1024 indices per call, 4 calls per
2048-edge group. int16 indices require <=32768-row table windows, so
edges are bucketed host-side by (src_chunk, tgt_chunk) over 4 chunks of
25088 atoms; each bucket has a static quota of 8 groups (16384 edges),
padded with index-0 edges. Within a bucket edges are Morton-sorted for
DRAM row locality. The host unpermutes the output rows.

Pair table Tp[a] = [T1[a] | T2[a]] (f32, [100352, 128]) is built on
device (phase 1) and gathered with elem_step=128: src reads cols 0:64
of row src_local, tgt reads cols 64:128. h/W/m are fed bf16 (f32 psum).
"""

import numpy as np

import concourse.bass as bass
import concourse.mybir as mybir
from concourse import bacc
from concourse.tile import TileContext
from concourse.bass_utils import run_bass_kernel_spmd

N_CORES = 8
NUM_ATOMS = 100000
CH_A = 25088              # atoms per chunk
NCH = 4                   # chunks
A2 = CH_A * NCH           # 100352 padded atoms
E_CORE = 250000
NB = 16                   # buckets = src_chunk * 4 + tgt_chunk
BG = 8                    # groups per bucket
QUOTA = BG * 2048         # 16384 edges per bucket
NG2 = NB * BG             # 128 groups per core
E_DEV = NG2 * 2048        # 262144 slots
SCALE = 1.0 / 0.6
F32 = mybir.dt.float32
BF16 = mybir.dt.bfloat16
I16 = mybir.dt.int16
NP_BF16 = mybir.dt.np(BF16)

_PROG = None


def _build_program():
    nc = bacc.Bacc("TRN2", target_bir_lowering=False, debug=False,
                   num_swdge_queues=4)
    hT = nc.dram_tensor("hT", [64, A2], BF16, kind="ExternalInput")
    Wcat = nc.dram_tensor("Wcat", [64, 128], BF16, kind="ExternalInput")
    W3 = nc.dram_tensor("W3", [16, 64], BF16, kind="ExternalInput")
    idxw = nc.dram_tensor("idxw", [NG2, 128, 4, 64], I16, kind="ExternalInput")
    m_st = nc.dram_tensor("m_st", [NG2, 16, 2048], BF16, kind="ExternalInput")
    out = nc.dram_tensor("out", [NG2, 128, 16, 64], F32, kind="ExternalOutput")

    with TileContext(nc) as tc:
        with tc.tile_pool(name="dram", bufs=1, space="DRAM") as dpool:
            Tp = dpool.tile([A2, 128], F32)

            # ---- phase 1: pair table, 784 blocks of 128 atoms ----
            with tc.tile_pool(name="ph1", bufs=2) as p1, \
                 tc.tile_pool(name="cp", bufs=3) as cp, \
                 tc.tile_pool(name="ps1", bufs=4, space="PSUM") as ps1, \
                 tc.tile_pool(name="wp", bufs=1) as wp:
                wcat_sb = wp.tile([64, 128], BF16)
                nc.sync.dma_start(wcat_sb[:, :], Wcat[:, :])
                for c in range(A2 // 2048):
                    hTc = p1.tile([64, 2048], BF16, tag="hTc")
                    nc.sync.dma_start(hTc[:, :], hT[:, 2048 * c:2048 * (c + 1)])
                    for b in range(16):
                        ps = ps1.tile([128, 128], F32)
                        nc.tensor.matmul(
                            out=ps[:, :],
                            lhsT=hTc[:, 128 * b:128 * b + 128],
                            rhs=wcat_sb[:, :],
                            start=True, stop=True)
                        t12 = cp.tile([128, 128], F32, tag="t12")
                        nc.scalar.copy(t12[:, :], ps[:, :])
                        a0 = 2048 * c + 128 * b
                        nc.sync.dma_start(Tp[a0:a0 + 128, :], t12[:, :])

            tc.strict_bb_all_engine_barrier()

            # ---- phase 2: 128 groups of 2048 bucket-sorted edges ----
            with tc.tile_pool(name="ip", bufs=3) as ip, \
                 tc.tile_pool(name="mp", bufs=3) as mp, \
                 tc.tile_pool(name="gp", bufs=2) as gp, \
                 tc.tile_pool(name="vp", bufs=4) as vp, \
                 tc.tile_pool(name="op", bufs=2) as op, \
                 tc.tile_pool(name="ps2", bufs=4, space="PSUM") as ps2, \
                 tc.tile_pool(name="wp2", bufs=1) as wp2:
                w3_sb = wp2.tile([16, 64], BF16)
                nc.sync.dma_start(w3_sb[:, :], W3[:, :])
                for t in range(NG2):
                    bs, bt = (t // BG) >> 2, (t // BG) & 3
                    src_tab = Tp[CH_A * bs:CH_A * (bs + 1), 0:64]
                    tgt_tab = Tp[CH_A * bt:CH_A * (bt + 1), 64:128]
                    it = ip.tile([128, 4, 64], I16, tag="it")
                    nc.sync.dma_start(it[:, :, :], idxw[t])
                    mst = mp.tile([16, 2048], BF16, tag="mst")
                    nc.sync.dma_start(mst[:, :], m_st[t])
                    gs = gp.tile([128, 16, 64], F32, tag="gs")
                    gt2 = gp.tile([128, 16, 64], F32, tag="gt")
                    for half in range(2):
                        nc.gpsimd.dma_gather(
                            out_ap=gs[:, 8 * half:8 * half + 8, :],
                            in_ap=src_tab,
                            idxs_ap=it[:, half, :],
                            num_idxs=1024, num_idxs_reg=1024,
                            elem_size=64, elem_step=128,
                            transpose=False, single_packet=False,
                            queue_num=(4 * t + 2 * half) % 4)
                        nc.gpsimd.dma_gather(
                            out_ap=gt2[:, 8 * half:8 * half + 8, :],
                            in_ap=tgt_tab,
                            idxs_ap=it[:, 2 + half, :],
                            num_idxs=1024, num_idxs_reg=1024,
                            elem_size=64, elem_step=128,
                            transpose=False, single_packet=False,
                            queue_num=(4 * t + 2 * half + 1) % 4)
                    ot = op.tile([128, 16, 64], F32, tag="ot")
                    for half in range(2):
                        ps = ps2.tile([128, 8, 64], F32)
                        for j8 in range(8):
                            j = 8 * half + j8
                            nc.tensor.matmul(
                                out=ps[:, j8, :],
                                lhsT=mst[:, 128 * j:128 * j + 128],
                                rhs=w3_sb[:, :],
                                start=True, stop=True)
                        y = vp.tile([128, 8, 64], F32, tag="y")
                        nc.vector.tensor_tensor(
                            out=y[:, :, :],
                            in0=gs[:, 8 * half:8 * half + 8, :],
                            in1=gt2[:, 8 * half:8 * half + 8, :],
                            op=mybir.AluOpType.add)
                        nc.vector.tensor_tensor(
                            out=y[:, :, :], in0=y[:, :, :], in1=ps[:, :, :],
                            op=mybir.AluOpType.add)
                        z = vp.tile([128, 8, 64], F32, tag="z")
                        nc.scalar.activation(
                            out=z[:, :, :], in_=y[:, :, :],
                            func=mybir.ActivationFunctionType.Silu)
                        nc.vector.tensor_scalar_mul(
                            ot[:, 8 * half:8 * half + 8, :], z[:, :, :], SCALE)
                    nc.scalar.dma_start(out[t], ot[:, :, :])
    nc.finalize()
    return nc


def _part1by1(x):
    x = x.astype(np.uint32)
    x &= 0x0000ffff
    x = (x | (x << 8)) & 0x00ff00ff
    x = (x | (x << 4)) & 0x0f0f0f0f
    x = (x | (x << 2)) & 0x33333333
    x = (x | (x << 1)) & 0x55555555
    return x


def _morton(x, y):
    return _part1by1(x) | (_part1by1(y) << 1)


def _wrap_idx(vals):
    """[NG2*2048] int16 -> [NG2, 128, 2, 64]: per 1024-idx call, index k
    lives at partition k%16 (replicated x8), free position k//16."""
    v = vals.reshape(NG2, 2, 64, 16)           # [t, call, fpos, pmod]
    v = v.transpose(0, 3, 1, 2)                # [t, pmod, call, fpos]
    return np.tile(v, (1, 8, 1, 1))            # [t, 128, 2, 64]


def _prepare_inputs(h, m, edge_index, W):
    h = np.asarray(h, dtype=np.float32)
    m = np.asarray(m, dtype=np.float32)
    W = np.asarray(W, dtype=np.float32)
    ei = np.asarray(edge_index).astype(np.int64)

    hT = np.zeros((64, A2), dtype=NP_BF16)
    hT[:, :NUM_ATOMS] = h.T.astype(NP_BF16)
    Wcat = np.concatenate([W[0:64, :], W[64:128, :]], axis=1).astype(NP_BF16)
    W3 = W[128:144, :].astype(NP_BF16)

    in_maps = []
    perms = []
    for c in range(N_CORES):
        lo = c * E_CORE
        src = ei[0, lo:lo + E_CORE]
        tgt = ei[1, lo:lo + E_CORE]
        bs = src // CH_A
        bt = tgt // CH_A
        bucket = bs * 4 + bt
        ls = (src - bs * CH_A).astype(np.int16)
        lt = (tgt - bt * CH_A).astype(np.int16)
        zkey = _morton(ls.astype(np.uint32), lt.astype(np.uint32))
        order = np.lexsort((zkey, bucket))
        counts = np.bincount(bucket, minlength=NB)
        if counts.max() > QUOTA:
            raise RuntimeError(f"bucket overflow: {counts.max()} > {QUOTA}")
        starts = np.concatenate([[0], np.cumsum(counts)])

        perm = np.full(E_DEV, -1, dtype=np.int64)   # slot -> edge id (core-local)
        ls_s = np.zeros(E_DEV, dtype=np.int16)
        lt_s = np.zeros(E_DEV, dtype=np.int16)
        m_s = np.zeros((E_DEV, 16), dtype=np.float32)
        for b in range(NB):
            ids = order[starts[b]:starts[b + 1]]
            base = b * QUOTA
            perm[base:base + len(ids)] = ids
            ls_s[base:base + len(ids)] = ls[ids]
            lt_s[base:base + len(ids)] = lt[ids]
            m_s[base:base + len(ids)] = m[lo + ids]

        idxw = np.concatenate([_wrap_idx(ls_s), _wrap_idx(lt_s)], axis=2)
        mst = np.ascontiguousarray(
            m_s.reshape(NG2, 16, 128, 16).transpose(0, 3, 1, 2)
               .reshape(NG2, 16, 2048)).astype(NP_BF16)
        in_maps.append({"hT": hT, "Wcat": Wcat, "W3": W3,
                        "idxw": np.ascontiguousarray(idxw), "m_st": mst})
        perms.append(perm)
    return in_maps, perms


def _run(inputs, trace=False):
    global _PROG
    if _PROG is None:
        _PROG = _build_program()
    in_maps, perms = _prepare_inputs(**inputs)
    res = run_bass_kernel_spmd(
        _PROG, in_maps, core_ids=list(range(N_CORES)), trace=trace)
    full = np.empty((N_CORES * E_CORE, 64), dtype=np.float32)
    for c in range(N_CORES):
        o = res.results[c]["out"]  # [NG2, 128, 16, 64]
        o = o.transpose(0, 2, 1, 3).reshape(E_DEV, 64)
        perm = perms[c]
        valid = perm >= 0
        full[c * E_CORE + perm[valid]] = o[valid]
    return full, res


def kernel(h, m, edge_index, W):
    full, _ = _run(dict(h=h, m=m, edge_index=edge_index, W=W), trace=False)
    return full


# revision 5
# speedup vs baseline: 1.4974x; 1.0851x over previous
"""EdgeEmbedding kernel for 8 Trainium2 NeuronCores, v3.

y[e] = silu(concat(h[src[e]], h[tgt[e]], m[e]) @ W) / 0.6
     = silu(T1[src] + T2[tgt] + m @ W3) / 0.6,  T1|T2 = h @ [W1|W2]

v3: replaces the per-128-descriptor indirect_dma_start gathers (Pool
engine 65% busy on ~1us/call SWDGE fixed cost in the baseline) with
dma_gather (InstDMAGatherAnt): 1024 indices per call, 4 calls per
2048-edge group. int16 indices require <=32768-row table windows, so
edges are bucketed host-side by (src_chunk, tgt_chunk) over 4 chunks of
25088 atoms; each bucket has a static quota of 8 groups (16384 edges),
padded with index-0 edges. Within a bucket edges are Morton-sorted for
DRAM row locality. The host unpermutes the output rows.

Pair table Tp[a] = [T1[a] | T2[a]] (f32, [100352, 128]) is built on
device (phase 1) and gathered with elem_step=128: src reads cols 0:64
of row src_local, tgt reads cols 64:128. h/W/m are fed bf16 (f32 psum).
"""

import numpy as np

import concourse.bass as bass
import concourse.mybir as mybir
from concourse import bacc
from concourse.tile import TileContext
from concourse.bass_utils import run_bass_kernel_spmd

N_CORES = 8
NUM_ATOMS = 100000
CH_A = 25088              # atoms per chunk
NCH = 4                   # chunks
A2 = CH_A * NCH           # 100352 padded atoms
E_CORE = 250000
NB = 16                   # buckets = src_chunk * 4 + tgt_chunk
BG = 8                    # groups per bucket
QUOTA = BG * 2048         # 16384 edges per bucket
NG2 = NB * BG             # 128 groups per core
E_DEV = NG2 * 2048        # 262144 slots
SCALE = 1.0 / 0.6
F32 = mybir.dt.float32
BF16 = mybir.dt.bfloat16
I16 = mybir.dt.int16
NP_BF16 = mybir.dt.np(BF16)

_PROG = None


def _build_program():
    nc = bacc.Bacc("TRN2", target_bir_lowering=False, debug=False,
                   num_swdge_queues=4)
    hT = nc.dram_tensor("hT", [64, A2], BF16, kind="ExternalInput")
    Wcat = nc.dram_tensor("Wcat", [64, 128], BF16, kind="ExternalInput")
    W3 = nc.dram_tensor("W3", [16, 64], BF16, kind="ExternalInput")
    idxw = nc.dram_tensor("idxw", [NG2, 128, 4, 64], I16, kind="ExternalInput")
    m_st = nc.dram_tensor("m_st", [NG2, 16, 2048], BF16, kind="ExternalInput")
    out = nc.dram_tensor("out", [NG2, 128, 16, 64], F32, kind="ExternalOutput")

    with TileContext(nc) as tc:
        with tc.tile_pool(name="dram", bufs=1, space="DRAM") as dpool:
            Tp = dpool.tile([A2, 128], F32)

            # ---- phase 1: pair table, 784 blocks of 128 atoms ----
            with tc.tile_pool(name="ph1", bufs=3) as p1, \
                 tc.tile_pool(name="cp", bufs=4) as cp, \
                 tc.tile_pool(name="ps1", bufs=4, space="PSUM") as ps1, \
                 tc.tile_pool(name="wp", bufs=1) as wp:
                wcat_sb = wp.tile([64, 128], BF16)
                nc.sync.dma_start(wcat_sb[:, :], Wcat[:, :])
                for c in range(A2 // 2048):
                    hTc = p1.tile([64, 2048], BF16, tag="hTc")
                    nc.sync.dma_start(hTc[:, :], hT[:, 2048 * c:2048 * (c + 1)])
                    for q in range(4):
                        ps = ps1.tile([128, 4, 128], F32)
                        for b in range(4):
                            nc.tensor.matmul(
                                out=ps[:, b, :],
                                lhsT=hTc[:, 512 * q + 128 * b:
                                         512 * q + 128 * b + 128],
                                rhs=wcat_sb[:, :],
                                start=True, stop=True)
                        t12 = cp.tile([128, 4, 128], F32, tag="t12")
                        nc.scalar.copy(t12[:, :, :], ps[:, :, :])
                        for b in range(4):
                            a0 = 2048 * c + 512 * q + 128 * b
                            eng = nc.sync if b % 2 == 0 else nc.scalar
                            eng.dma_start(Tp[a0:a0 + 128, :], t12[:, b, :])

            tc.strict_bb_all_engine_barrier()

            # ---- phase 2: 128 groups of 2048 bucket-sorted edges ----
            with tc.tile_pool(name="ip", bufs=3) as ip, \
                 tc.tile_pool(name="mp", bufs=3) as mp, \
                 tc.tile_pool(name="gp", bufs=2) as gp, \
                 tc.tile_pool(name="vp", bufs=4) as vp, \
                 tc.tile_pool(name="op", bufs=2) as op, \
                 tc.tile_pool(name="ps2", bufs=4, space="PSUM") as ps2, \
                 tc.tile_pool(name="wp2", bufs=1) as wp2:
                w3_sb = wp2.tile([16, 64], BF16)
                nc.sync.dma_start(w3_sb[:, :], W3[:, :])
                for t in range(NG2):
                    bs, bt = (t // BG) >> 2, (t // BG) & 3
                    src_tab = Tp[CH_A * bs:CH_A * (bs + 1), 0:64]
                    tgt_tab = Tp[CH_A * bt:CH_A * (bt + 1), 64:128]
                    it = ip.tile([128, 4, 64], I16, tag="it")
                    nc.sync.dma_start(it[:, :, :], idxw[t])
                    mst = mp.tile([16, 2048], BF16, tag="mst")
                    nc.sync.dma_start(mst[:, :], m_st[t])
                    gs = gp.tile([128, 16, 64], F32, tag="gs")
                    gt2 = gp.tile([128, 16, 64], F32, tag="gt")
                    for half in range(2):
                        nc.gpsimd.dma_gather(
                            out_ap=gs[:, 8 * half:8 * half + 8, :],
                            in_ap=src_tab,
                            idxs_ap=it[:, half, :],
                            num_idxs=1024, num_idxs_reg=1024,
                            elem_size=64, elem_step=128,
                            transpose=False, single_packet=False,
                            queue_num=(4 * t + 2 * half) % 4)
                        nc.gpsimd.dma_gather(
                            out_ap=gt2[:, 8 * half:8 * half + 8, :],
                            in_ap=tgt_tab,
                            idxs_ap=it[:, 2 + half, :],
                            num_idxs=1024, num_idxs_reg=1024,
                            elem_size=64, elem_step=128,
                            transpose=False, single_packet=False,
                            queue_num=(4 * t + 2 * half + 1) % 4)
                    ot = op.tile([128, 16, 64], F32, tag="ot")
                    for half in range(2):
                        ps = ps2.tile([128, 8, 64], F32)
                        for j8 in range(8):
                            j = 8 * half + j8
                            nc.tensor.matmul(
                                out=ps[:, j8, :],
                                lhsT=mst[:, 128 * j:128 * j + 128],
                                rhs=w3_sb[:, :],
                                start=True, stop=True)
                        y = vp.tile([128, 8, 64], F32, tag="y")
                        nc.vector.tensor_tensor(
                            out=y[:, :, :],
                            in0=gs[:, 8 * half:8 * half + 8, :],
                            in1=gt2[:, 8 * half:8 * half + 8, :],
                            op=mybir.AluOpType.add)
                        nc.vector.tensor_tensor(
                            out=y[:, :, :], in0=y[:, :, :], in1=ps[:, :, :],
                            op=mybir.AluOpType.add)
                        z = vp.tile([128, 8, 64], F32, tag="z")
                        nc.scalar.activation(
                            out=z[:, :, :], in_=y[:, :, :],
                            func=mybir.ActivationFunctionType.Silu)
                        nc.vector.tensor_scalar_mul(
                            ot[:, 8 * half:8 * half + 8, :], z[:, :, :], SCALE)
                    nc.scalar.dma_start(out[t], ot[:, :, :])
    nc.finalize()
    return nc


def _part1by1(x):
    x = x.astype(np.uint32)
    x &= 0x0000ffff
    x = (x | (x << 8)) & 0x00ff00ff
    x = (x | (x << 4)) & 0x0f0f0f0f
    x = (x | (x << 2)) & 0x33333333
    x = (x | (x << 1)) & 0x55555555
    return x


def _morton(x, y):
    return _part1by1(x) | (_part1by1(y) << 1)


def _wrap_idx(vals):
    """[NG2*2048] int16 -> [NG2, 128, 2, 64]: per 1024-idx call, index k
    lives at partition k%16 (replicated x8), free position k//16."""
    v = vals.reshape(NG2, 2, 64, 16)           # [t, call, fpos, pmod]
    v = v.transpose(0, 3, 1, 2)                # [t, pmod, call, fpos]
    return np.tile(v, (1, 8, 1, 1))            # [t, 128, 2, 64]


def _prepare_inputs(h, m, edge_index, W):
    h = np.asarray(h, dtype=np.float32)
    m = np.asarray(m, dtype=np.float32)
    W = np.asarray(W, dtype=np.float32)
    ei = np.asarray(edge_index).astype(np.int64)

    hT = np.zeros((64, A2), dtype=NP_BF16)
    hT[:, :NUM_ATOMS] = h.T.astype(NP_BF16)
    Wcat = np.concatenate([W[0:64, :], W[64:128, :]], axis=1).astype(NP_BF16)
    W3 = W[128:144, :].astype(NP_BF16)

    in_maps = []
    perms = []
    for c in range(N_CORES):
        lo = c * E_CORE
        src = ei[0, lo:lo + E_CORE]
        tgt = ei[1, lo:lo + E_CORE]
        bs = src // CH_A
        bt = tgt // CH_A
        bucket = bs * 4 + bt
        ls = (src - bs * CH_A).astype(np.int16)
        lt = (tgt - bt * CH_A).astype(np.int16)
        zkey = _morton(ls.astype(np.uint32), lt.astype(np.uint32))
        order = np.lexsort((zkey, bucket))
        counts = np.bincount(bucket, minlength=NB)
        if counts.max() > QUOTA:
            raise RuntimeError(f"bucket overflow: {counts.max()} > {QUOTA}")
        starts = np.concatenate([[0], np.cumsum(counts)])

        perm = np.full(E_DEV, -1, dtype=np.int64)   # slot -> edge id (core-local)
        ls_s = np.zeros(E_DEV, dtype=np.int16)
        lt_s = np.zeros(E_DEV, dtype=np.int16)
        m_s = np.zeros((E_DEV, 16), dtype=np.float32)
        for b in range(NB):
            ids = order[starts[b]:starts[b + 1]]
            base = b * QUOTA
            perm[base:base + len(ids)] = ids
            ls_s[base:base + len(ids)] = ls[ids]
            lt_s[base:base + len(ids)] = lt[ids]
            m_s[base:base + len(ids)] = m[lo + ids]

        idxw = np.concatenate([_wrap_idx(ls_s), _wrap_idx(lt_s)], axis=2)
        mst = np.ascontiguousarray(
            m_s.reshape(NG2, 16, 128, 16).transpose(0, 3, 1, 2)
               .reshape(NG2, 16, 2048)).astype(NP_BF16)
        in_maps.append({"hT": hT, "Wcat": Wcat, "W3": W3,
                        "idxw": np.ascontiguousarray(idxw), "m_st": mst})
        perms.append(perm)
    return in_maps, perms


def _run(inputs, trace=False):
    global _PROG
    if _PROG is None:
        _PROG = _build_program()
    in_maps, perms = _prepare_inputs(**inputs)
    res = run_bass_kernel_spmd(
        _PROG, in_maps, core_ids=list(range(N_CORES)), trace=trace)
    full = np.empty((N_CORES * E_CORE, 64), dtype=np.float32)
    for c in range(N_CORES):
        o = res.results[c]["out"]  # [NG2, 128, 16, 64]
        o = o.transpose(0, 2, 1, 3).reshape(E_DEV, 64)
        perm = perms[c]
        valid = perm >= 0
        full[c * E_CORE + perm[valid]] = o[valid]
    return full, res


def kernel(h, m, edge_index, W):
    full, _ = _run(dict(h=h, m=m, edge_index=edge_index, W=W), trace=False)
    return full
